# revision 1
# baseline (speedup 1.0000x reference)
import numpy as np
import jax
import jax.numpy as jnp

# nn_MAB: B=256, Npt=25, Sd=10, T=40, C=64, inter=16, D=2560, 8 heads.
# Pure data parallel: batch 256 -> 32 per core across 8 NeuronCores.
# All tensors kept "v-major" (B, V, C, T) so gcn input/output are reshapes
# of the (B, V, C*T) attention layout — no large transposes on device.

NUM_SUBSET = 3
BN_EPS = 1e-5
T_CONST = 40
NUM_HEADS = 8
NCORES = 8


def _unit_gcn_v(x_v, PA, Wa, ba, Wb, bb, Wd, bd, gamma, beta):
    # x_v: (B, V, C, T)
    B, V, C, T = x_v.shape
    inter = Wa.shape[1]
    y = None
    for i in range(NUM_SUBSET):
        a = jnp.einsum('bvct,ic->bvit', x_v, Wa[i]) + ba[i][None, None, :, None]
        b = jnp.einsum('bvct,ic->bvit', x_v, Wb[i]) + bb[i][None, None, :, None]
        M = jnp.einsum('bvit,bwit->bvw', a, b) / (inter * T)
        S = jax.nn.softmax(M, axis=-2) + PA[i]          # (B, V, W): softmax over v
        z = jnp.einsum('bvw,bvct->bwct', S, x_v)        # (B, W, C, T)
        z = jnp.einsum('bwct,oc->bwot', z, Wd[i]) + bd[i][None, None, :, None]
        y = z if y is None else y + z
    y = y * (gamma / jnp.sqrt(1.0 + BN_EPS))[None, None, :, None] + beta[None, None, :, None]
    y = y + x_v
    return jax.nn.relu(y)


def _mab_forward(Q, K, fck, fcv, fco):
    B, Npt, DK = K.shape
    T = T_CONST
    C = DK // T
    Kv = K.reshape(B, Npt, C, T)
    Kg = _unit_gcn_v(Kv, *fck)
    Vg = _unit_gcn_v(Kv, *fcv)
    Kf = Kg.reshape(B, Npt, DK)
    Vf = Vg.reshape(B, Npt, DK)
    S, DV = Q.shape[1], Q.shape[2]
    ds = DV // NUM_HEADS
    Qh = Q.reshape(B, S, NUM_HEADS, ds)
    Kh = Kf.reshape(B, Npt, NUM_HEADS, ds)
    Vh = Vf.reshape(B, Npt, NUM_HEADS, ds)
    scores = jnp.einsum('bqhd,bkhd->bhqk', Qh, Kh) / jnp.sqrt(jnp.float32(DV))
    attn = jax.nn.softmax(scores, axis=-1)
    Oh = Qh + jnp.einsum('bhqk,bkhd->bqhd', attn, Vh)
    O = Oh.reshape(B, S, DV)
    Ov = O.reshape(B, S, C, T)
    Og = _unit_gcn_v(Ov, *fco)
    Og = Og.reshape(B, S, DK)
    return O + jax.nn.relu(Og)


_FCK = ('PA', 'Wa', 'ba', 'Wb', 'bb', 'Wd', 'bd', 'gamma', 'beta')


def _shard_fn(Q, K, params):
    fck = tuple(params['fck_' + n] for n in _FCK)
    fcv = tuple(params['fcv_' + n] for n in _FCK)
    fco = tuple(params['fco_' + n] for n in _FCK)
    return _mab_forward(Q, K, fck, fcv, fco)


_pmapped = None


def _get_pmapped():
    global _pmapped
    if _pmapped is None:
        _pmapped = jax.pmap(_shard_fn, in_axes=(0, 0, None), devices=jax.devices()[:NCORES])
    return _pmapped


def kernel(**inputs):
    Q = np.asarray(inputs['Q'], np.float32)
    K = np.asarray(inputs['K'], np.float32)
    B = Q.shape[0]
    params = {k: jnp.asarray(v) for k, v in inputs.items()
              if k.startswith(('fck_', 'fcv_', 'fco_'))}
    per = B // NCORES
    Qs = Q.reshape(NCORES, per, Q.shape[1], Q.shape[2])
    Ks = K.reshape(NCORES, per, K.shape[1], K.shape[2])
    out = _get_pmapped()(Qs, Ks, params)
    out = np.asarray(out)
    return out.reshape(B, out.shape[2], out.shape[3]).astype(np.float32)

